# revision 18
# baseline (speedup 1.0000x reference)
"""ESIM-style local inference modeling kernel for Trainium2 (Bass/Tile).

Problem (per batch item, B=32, La=Lb=512, D=768, fp32):
    E       = A @ B^T                      [512, 512]
    a_tilde = softmax(E, axis=1) @ B       [512, 768]
    b_tilde = softmax(E, axis=0)^T @ A     [512, 768]
    m_a     = concat([A, a_tilde, A - a_tilde, A * a_tilde], -1)
    m_b     = concat([B, b_tilde, B - b_tilde, B * b_tilde], -1)

Sharding: pure data-parallel, 4 batch items per core across 8 cores.

Strategy (v3): the device computes ONLY a_tilde / b_tilde (bf16); the
concat blocks are assembled host-side in fp32 from the exact fp32
inputs and the bf16 tildes.  fp8e4m3 DoubleRow matmuls (0.5
cycles/col, 256-deep contraction) carry both E and the a-side
attention:

  - E = A8'B8 + dA8'B8 + A8'dB8 from host-pre-transposed fp8 pairs
    (dX8 = fp8(X - fp8(X)), first-order residual).  More accurate
    than a bf16 E (logit rms 0.05 vs 0.055) at 0.75x the cycles, and
    no on-chip transposes.
  - a-side: weights pre-normalized into fp8 - Wa8 = fp8(U * (1/s1))
    lies in (0, 1] so fp8's dynamic range holds it; the ones column
    re-derives the denominator from the QUANTIZED weights, so fp8
    rounding of the dominant weight cancels in the ratio.  Values are
    the fp8 pair (B8, dB8).  UT8 = PE-transpose of Wa8.
    4 DR matmuls per group instead of 8 bf16 ones: half the PE time.
  - b-side stays bf16 (U x A_bf16): raw exp values span e^(+-65), far
    beyond fp8 range, and per-column rescaling is not partition-native.
    Measured end-to-end rel err 6.4e-3 vs the 2e-2 gate.
  - Normalize split: Act scales cols 1:512, DVE cols 512:769 (parallel,
    PSUM ring frees sooner).  Reciprocal AFTER chunk2: emitted between
    the chunks it stalls chunk2 on a tile-level WAR hazard.

Schedule: loads issue up front on SP in consumption-deadline order.
Item 0's four E tiles run piece-major across 4 live PSUM tiles
(2 epsum + 2 borrowed apsum).  Steady state: item i+1's E tiles+exps
ride between item i's a-side groups; item i's UT8 thunks ride between
its own b-side groups (after its Wa8 pass completes).  Stores for
items 0-1 go out on the Pool/SWDGE queue (SP is still dispatching
loads); items 2-3 store via SP/HWDGE.

Per-core HBM: in 3.14MB/item (A bf16 + B8/dB8 + 4 transposed fp8),
out 1.57MB/item = 18.9MB (~52us at 360GB/s aggregate) - DMA and PE
(~50us) are balanced at the ridge.
"""

import os as _os

import numpy as np

B, L, D = 32, 512, 768
NCORES = 8
BPC = B // NCORES          # batch items per core
NT = L // 128              # 4 row tiles per matrix
KD = D // 128              # 6 contraction chunks over d
DX = D + 1                 # attention rhs: col 0 = ones, cols 1..768 = data
N1 = 512                   # attention chunk 1: psum cols [s | out 0..510]
C_SHIFT = 120.0            # softmax stabilization shift (valid ~[100, 142])

E_FP8 = int(_os.environ.get("K_E_FP8", "1"))   # E via fp8 DoubleRow + residual
WARMUP = int(_os.environ.get("K_WARMUP", "6"))
EP_BUFS = int(_os.environ.get("K_EP", "2"))
TP_BUFS = int(_os.environ.get("K_TP", "2"))
AP_BUFS = int(_os.environ.get("K_AP", "2"))
AP2_BUFS = int(_os.environ.get("K_AP2", "2"))
OUTP_BUFS = int(_os.environ.get("K_OUTP", "8"))
POOL_STORE_N = int(_os.environ.get("K_PSTORE", "1"))  # items stored via SWDGE
WA8_POOL = int(_os.environ.get("K_WA8POOL", "1"))  # Wa8 scale pass on GpSimd

_CACHE: dict = {}


def _build_bass():
    from contextlib import ExitStack

    import concourse.mybir as mybir
    import concourse.tile as tile
    from concourse import bacc
    from concourse.masks import make_identity

    f32 = mybir.dt.float32
    bf16 = mybir.dt.bfloat16
    f8 = mybir.dt.float8e4
    DR = mybir.MatmulPerfMode.DoubleRow

    nc = bacc.Bacc("TRN2", target_bir_lowering=False, debug=False)

    a_in = nc.dram_tensor("a", [BPC, L, D], bf16, kind="ExternalInput").ap()
    b8u_in = nc.dram_tensor("b8u", [BPC, L, D], f8, kind="ExternalInput").ap()
    db8u_in = nc.dram_tensor("db8u", [BPC, L, D], f8, kind="ExternalInput").ap()
    if E_FP8:
        ha8 = nc.dram_tensor("ha8", [BPC, D, L], f8, kind="ExternalInput").ap()
        hda8 = nc.dram_tensor("hda8", [BPC, D, L], f8, kind="ExternalInput").ap()
        hb8 = nc.dram_tensor("hb8", [BPC, D, L], f8, kind="ExternalInput").ap()
        hdb8 = nc.dram_tensor("hdb8", [BPC, D, L], f8, kind="ExternalInput").ap()
    else:
        ha = nc.dram_tensor("ha", [BPC, D, L], bf16, kind="ExternalInput").ap()
        hb = nc.dram_tensor("hb", [BPC, D, L], bf16, kind="ExternalInput").ap()
    mat_out = nc.dram_tensor("mat", [BPC, L, D], bf16, kind="ExternalOutput").ap()
    mbt_out = nc.dram_tensor("mbt", [BPC, L, D], bf16, kind="ExternalOutput").ap()

    with tile.TileContext(nc) as tc, ExitStack() as ctx:
        singles = ctx.enter_context(tc.tile_pool(name="singles", bufs=1))
        inp = ctx.enter_context(tc.tile_pool(name="inp", bufs=BPC))
        hat = ctx.enter_context(tc.tile_pool(name="hat", bufs=2))
        usb = ctx.enter_context(tc.tile_pool(name="usb", bufs=3))
        outp = ctx.enter_context(tc.tile_pool(name="outp", bufs=OUTP_BUFS))
        stats = ctx.enter_context(tc.tile_pool(name="stats", bufs=16))
        epsum = ctx.enter_context(tc.tile_pool(name="epsum", bufs=EP_BUFS, space="PSUM"))
        tpsum = ctx.enter_context(tc.tile_pool(name="tpsum", bufs=TP_BUFS, space="PSUM"))
        apsum1 = ctx.enter_context(tc.tile_pool(name="apsum1", bufs=AP_BUFS, space="PSUM"))
        apsum2 = ctx.enter_context(tc.tile_pool(name="apsum2", bufs=AP2_BUFS, space="PSUM"))

        ident_f = singles.tile([128, 128], f32, tag="ident_f")
        make_identity(nc, ident_f)
        ident = singles.tile([128, 128], bf16, tag="ident")
        nc.scalar.copy(ident, ident_f)
        neg_shift = singles.tile([128, 1], f32, tag="neg_shift")
        nc.vector.memset(neg_shift, -C_SHIFT)

        # ---- PE p-state warmup: independent of ident (which rides a slow
        # gpsimd iota chain) so it starts immediately.
        if WARMUP:
            wident = singles.tile([128, 128], bf16, tag="wident")
            nc.vector.memset(wident, 0.0)
            wp = apsum1.tile([128, N1], f32, tag="c1")
            for w in range(WARMUP):
                nc.tensor.matmul(
                    wp[:, (w % 4) * 128:(w % 4) * 128 + 128],
                    lhsT=wident, rhs=wident,
                )

        # ---- tiles + load thunks per item (issued in deadline order below)
        inps, hats = [], []
        load_hats, load_ax, load_bx = [], [], []
        for i in range(BPC):
            AX = inp.tile([128, NT, DX], bf16, tag="AX", name=f"AX{i}")
            B8X = inp.tile([128, NT, DX], f8, tag="B8X", name=f"B8X{i}")
            DB8X = inp.tile([128, NT, DX], f8, tag="DB8X", name=f"DB8X{i}")
            nc.gpsimd.memset(AX[:, :, 0:1], 1.0)
            nc.gpsimd.memset(B8X[:, :, 0:1], 1.0)
            nc.gpsimd.memset(DB8X[:, :, 0:1], 0.0)
            if E_FP8:
                HA = hat.tile([128, KD, L], f8, tag="HA", name=f"HA{i}")
                HDA = hat.tile([128, KD, L], f8, tag="HDA", name=f"HDA{i}")
                HB = hat.tile([128, KD, L], f8, tag="HB", name=f"HB{i}")
                HDB = hat.tile([128, KD, L], f8, tag="HDB", name=f"HDB{i}")
                srcs = ((HA, ha8), (HB, hb8), (HDA, hda8), (HDB, hdb8))
            else:
                HA = hat.tile([128, KD, L], bf16, tag="HA", name=f"HA{i}")
                HB = hat.tile([128, KD, L], bf16, tag="HB", name=f"HB{i}")
                HDA = HDB = None
                srcs = ((HA, ha), (HB, hb))

            def mk_hats(i=i, srcs=srcs):
                if i == 0 and len(srcs) == 4:
                    # main term tensors whole; residuals in interleaved
                    # halves so the E residual matmuls track the arrivals
                    for dst, src in srcs[:2]:
                        nc.sync.dma_start(
                            out=dst,
                            in_=src[i].rearrange("(k p) l -> p k l", p=128),
                        )
                    for h in range(2):
                        for dst, src in srcs[2:]:
                            nc.sync.dma_start(
                                out=dst[:, 3 * h:3 * h + 3, :],
                                in_=src[i, 384 * h:384 * h + 384].rearrange(
                                    "(k p) l -> p k l", p=128
                                ),
                            )
                else:
                    for dst, src in srcs:
                        nc.sync.dma_start(
                            out=dst,
                            in_=src[i].rearrange("(k p) l -> p k l", p=128),
                        )

            def mk_ax(i=i, AX=AX, thirds=(i == 0)):
                src = a_in[i].rearrange("(t p) d -> p t d", p=128)
                if thirds:
                    bounds = [D * q // 3 for q in range(4)]
                    for q in range(3):
                        lo, hi = bounds[q], bounds[q + 1]
                        nc.sync.dma_start(
                            out=AX[:, :, 1 + lo:1 + hi], in_=src[:, :, lo:hi]
                        )
                else:
                    nc.sync.dma_start(out=AX[:, :, 1:DX], in_=src)

            def mk_bx(i=i, B8X=B8X, DB8X=DB8X):
                for dst, src in ((B8X, b8u_in), (DB8X, db8u_in)):
                    nc.sync.dma_start(
                        out=dst[:, :, 1:DX],
                        in_=src[i].rearrange("(t p) d -> p t d", p=128),
                    )

            load_hats.append(mk_hats)
            load_ax.append(mk_ax)
            load_bx.append(mk_bx)
            inps.append((AX, B8X, DB8X))
            hats.append((HA, HDA, HB, HDB))

        # deadline order: hats0, AX0, hats1, BX0, AX1, hats2, BX1, AX2,
        # hats3, BX2, AX3, BX3
        load_hats[0]()
        load_ax[0]()
        load_hats[1]()
        load_bx[0]()
        load_ax[1]()
        load_hats[2]()
        load_bx[1]()
        load_ax[2]()
        load_hats[3]()
        load_bx[2]()
        load_ax[3]()
        load_bx[3]()

        # ---- per-item tiles ----------------------------------------------
        Us, Wa8s, UT8s, s1ps, rs1s = [], [], [], [], []
        for i in range(BPC):
            Us.append(usb.tile([128, NT, L], bf16, tag="U", name=f"U{i}"))
            Wa8s.append(usb.tile([128, NT, L], f8, tag="Wa8", name=f"Wa8{i}"))
            UT8s.append(usb.tile([128, NT, L], f8, tag="UT8", name=f"UT8{i}"))
            s1ps.append(stats.tile([128, NT], f32, tag="s1p", name=f"s1p{i}"))
            rs1s.append(stats.tile([128, NT], f32, tag="rs1", name=f"rs1{i}"))

        def e_terms(i):
            HA, HDA, HB, HDB = hats[i]
            if E_FP8:
                return ((HA, HB), (HDA, HB), (HA, HDB))
            return ((HA, HB),)

        def e_matmul(pe, lt, rt, ta, kp, start, stop):
            if E_FP8:
                nc.tensor.matmul(
                    pe,
                    lhsT=lt[:, 2 * kp:2 * kp + 2, ta * 128:(ta + 1) * 128],
                    rhs=rt[:, 2 * kp:2 * kp + 2, :],
                    start=start, stop=stop, perf_mode=DR,
                    skip_group_check=True,
                )
            else:
                nc.tensor.matmul(
                    pe,
                    lhsT=lt[:, kp, ta * 128:(ta + 1) * 128],
                    rhs=rt[:, kp, :],
                    start=start, stop=stop,
                    skip_group_check=True,
                )

        NKP = (KD // 2) if E_FP8 else KD

        def e_exp(i, ta, pe, half=None):
            if half is None:
                lo, hi = 0, L
            else:
                lo, hi = half * (L // 2), (half + 1) * (L // 2)
            nc.scalar.activation(
                Us[i][:, ta, lo:hi], pe[:, lo:hi],
                mybir.ActivationFunctionType.Exp,
                bias=neg_shift, scale=1.0,
            )

        def wa8_tile(i, ta):
            """1/s1 for tile ta (DVE) -> Wa8 tile = fp8(U * rs1) (Act).
            Each partition holds a different logical row per ta tile, so
            the scale is per-ta."""
            nc.vector.reduce_sum(
                s1ps[i][:, ta:ta + 1], Us[i][:, ta, :],
                axis=mybir.AxisListType.X,
            )
            nc.vector.reciprocal(rs1s[i][:, ta:ta + 1], s1ps[i][:, ta:ta + 1])
            if WA8_POOL:
                nc.gpsimd.tensor_scalar_mul(
                    Wa8s[i][:, ta, :], Us[i][:, ta, :],
                    rs1s[i][:, ta:ta + 1],
                )
            else:
                nc.scalar.activation(
                    Wa8s[i][:, ta, :], Us[i][:, ta, :],
                    mybir.ActivationFunctionType.Copy,
                    scale=rs1s[i][:, ta:ta + 1],
                )

        def e_tile_thunk(i, ta):
            """One E tile (term-major) + exp, for steady-state stages."""
            def th():
                pe = epsum.tile([128, L], f32, tag="pe")
                terms = e_terms(i)
                n = len(terms) * NKP
                j = 0
                for lt, rt in terms:
                    for kp in range(NKP):
                        e_matmul(pe, lt, rt, ta, kp, j == 0, j == n - 1)
                        j += 1
                e_exp(i, ta, pe)
                wa8_tile(i, ta)
            return th

        def ut_thunk(i, tcq):
            def th():
                tp = tpsum.tile([128, L], f8, tag="tp")
                for ta in range(NT):
                    nc.tensor.transpose(
                        tp[:, ta * 128:(ta + 1) * 128],
                        Wa8s[i][:, ta, tcq * 128:(tcq + 1) * 128],
                        ident,
                    )
                nc.vector.tensor_copy(UT8s[i][:, tcq, :], tp)
            return th

        def attn_group(i, side, t):
            AX, B8X, DB8X = inps[i]
            out_dram = mbt_out if side == "b" else mat_out
            # two separate PSUM tiles: the reciprocal + Act normalize of
            # chunk1 overlap chunk2's matmuls (no shared-tile WAR), and
            # each ring frees as soon as its own norm has read it.
            c1 = apsum1.tile([128, N1], f32, tag="c1")
            c2 = apsum2.tile([128, DX - N1], f32, tag="c2")

            def chunk(dst, lo, hi):
                if side == "b":
                    for kc in range(NT):
                        nc.tensor.matmul(
                            dst,
                            lhsT=Us[i][:, kc, t * 128:(t + 1) * 128],
                            rhs=AX[:, kc, lo:hi],
                            start=(kc == 0), stop=(kc == NT - 1),
                        )
                else:
                    j = 0
                    for V in (B8X, DB8X):
                        for q in range(NT // 2):
                            nc.tensor.matmul(
                                dst,
                                lhsT=UT8s[i][:, 2 * q:2 * q + 2,
                                             t * 128:(t + 1) * 128],
                                rhs=V[:, 2 * q:2 * q + 2, lo:hi],
                                start=(j == 0), stop=(j == 3),
                                perf_mode=DR,
                            )
                            j += 1

            chunk(c1, 0, N1)
            r = stats.tile([128, 1], f32, tag="r")
            nc.vector.reciprocal(r, c1[:, 0:1])
            ot = outp.tile([128, D], bf16, tag="ot")
            nc.scalar.activation(
                ot[:, 0:N1 - 1], c1[:, 1:N1],
                mybir.ActivationFunctionType.Copy, scale=r,
            )
            chunk(c2, N1, DX)
            nc.vector.tensor_scalar_mul(ot[:, N1 - 1:D], c2, r)
            rows = slice(t * 128, (t + 1) * 128)
            q = nc.gpsimd if i < POOL_STORE_N else nc.sync
            q.dma_start(out=out_dram[i, rows, :], in_=ot)

        # ---- prologue: item 0's E piece-major across 4 live PSUM tiles ---
        e0 = [
            epsum.tile([128, L], f32, tag="pe", name="e0p0"),
            epsum.tile([128, L], f32, tag="pe", name="e0p1"),
            apsum1.tile([128, N1], f32, tag="c1", name="e0p2"),
            apsum1.tile([128, N1], f32, tag="c1", name="e0p3"),
        ]
        terms0 = e_terms(0)
        npiece = len(terms0) * NKP
        j = 0
        for lt, rt in terms0:
            for kp in range(NKP):
                for ta in range(NT):
                    e_matmul(e0[ta], lt, rt, ta, kp, j == 0, j == npiece - 1)
                j += 1
        # exps in halves: tiles 2,3 first (they hold the borrowed apsum
        # buffers the first b-groups need), then tile 0/1 halves in the
        # order the first b-group's lhsT slices want them.
        for ta, h in ((2, 0), (2, 1), (3, 0), (3, 1),
                      (0, 0), (1, 0), (0, 1), (1, 1)):
            e_exp(0, ta, e0[ta], half=h)
        for ta in range(NT):
            wa8_tile(0, ta)

        # ---- steady state -------------------------------------------------
        pend_ut = [ut_thunk(0, tcq) for tcq in range(NT)]
        for i in range(BPC):
            last = i == BPC - 1
            slots = (0, 0, 2, 2) if i == 0 else (0, 2, 2, 0)
            if not last:
                for t in range(NT):
                    attn_group(i, "b", t)
                    for _ in range(slots[t]):
                        if pend_ut:
                            pend_ut.pop(0)()
                nxt_e = [e_tile_thunk(i + 1, ta) for ta in range(NT)]
                for t in range(NT):
                    attn_group(i, "a", t)
                    if nxt_e:
                        nxt_e.pop(0)()
                pend_ut = [ut_thunk(i + 1, tcq) for tcq in range(NT)]
            else:
                # no fillers left: interleave a-groups into the b-half so
                # the 2-deep PSUM ring never starves PE; "u" slots run the
                # UT8 transposes the a-groups need.
                seq = [("b", 0), "u", "u", ("b", 1), "u", "u",
                       ("b", 2), ("a", 0), ("b", 3), ("a", 1),
                       ("a", 2), ("a", 3)]
                for ent in seq:
                    if ent == "u":
                        if pend_ut:
                            pend_ut.pop(0)()
                    else:
                        attn_group(i, ent[0], ent[1])

    nc.compile()
    return nc


def _get_nc():
    if "nc" not in _CACHE:
        _CACHE["nc"] = _build_bass()
    return _CACHE["nc"]


def host_prep(a_bar, b_bar):
    """Full-batch [B, L, D] fp32 -> per-input dram arrays (full batch)."""
    import ml_dtypes

    bf = ml_dtypes.bfloat16
    f8 = ml_dtypes.float8_e4m3
    a32 = np.asarray(a_bar, dtype=np.float32)
    b32 = np.asarray(b_bar, dtype=np.float32)
    a8 = a32.astype(f8)
    da8 = (a32 - a8.astype(np.float32)).astype(f8)
    b8 = b32.astype(f8)
    db8 = (b32 - b8.astype(np.float32)).astype(f8)
    out = {
        "a": np.ascontiguousarray(a32.astype(bf)),
        "b8u": np.ascontiguousarray(b8),
        "db8u": np.ascontiguousarray(db8),
    }
    if E_FP8:
        out["ha8"] = np.ascontiguousarray(a8.transpose(0, 2, 1))
        out["hda8"] = np.ascontiguousarray(da8.transpose(0, 2, 1))
        out["hb8"] = np.ascontiguousarray(b8.transpose(0, 2, 1))
        out["hdb8"] = np.ascontiguousarray(db8.transpose(0, 2, 1))
    else:
        out["ha"] = np.ascontiguousarray(
            a32.astype(bf).transpose(0, 2, 1)
        )
        out["hb"] = np.ascontiguousarray(
            b32.astype(bf).transpose(0, 2, 1)
        )
    return out


def assemble(x32, t_bf16):
    """m = concat([x, t, x - t, x * t], -1) in fp32."""
    n, l, d = x32.shape
    m = np.empty((n, l, 4 * d), dtype=np.float32)
    t = np.asarray(t_bf16, dtype=np.float32)
    m[:, :, 0:d] = x32
    m[:, :, d:2 * d] = t
    m[:, :, 2 * d:3 * d] = x32 - t
    m[:, :, 3 * d:4 * d] = x32 * t
    return m


def kernel(a_bar, b_bar):
    from concourse import bass_utils

    a32 = np.asarray(a_bar, dtype=np.float32)
    b32 = np.asarray(b_bar, dtype=np.float32)
    full = host_prep(a32, b32)
    nc = _get_nc()
    in_maps = []
    for r in range(NCORES):
        sl = slice(r * BPC, (r + 1) * BPC)
        in_maps.append({k: v[sl] for k, v in full.items()})
    res = bass_utils.run_bass_kernel_spmd(nc, in_maps, core_ids=list(range(NCORES)))

    at = np.concatenate(
        [np.asarray(res.results[r]["mat"]) for r in range(NCORES)], axis=0
    )
    bt = np.concatenate(
        [np.asarray(res.results[r]["mbt"]) for r in range(NCORES)], axis=0
    )
    return assemble(a32, at), assemble(b32, bt)


# revision 19
# speedup vs baseline: 1.0202x; 1.0202x over previous
"""ESIM-style local inference modeling kernel for Trainium2 (Bass/Tile).

Problem (per batch item, B=32, La=Lb=512, D=768, fp32):
    E       = A @ B^T                      [512, 512]
    a_tilde = softmax(E, axis=1) @ B       [512, 768]
    b_tilde = softmax(E, axis=0)^T @ A     [512, 768]
    m_a     = concat([A, a_tilde, A - a_tilde, A * a_tilde], -1)
    m_b     = concat([B, b_tilde, B - b_tilde, B * b_tilde], -1)

Sharding: pure data-parallel, 4 batch items per core across 8 cores.

Strategy (v3): the device computes ONLY a_tilde / b_tilde (bf16); the
concat blocks are assembled host-side in fp32 from the exact fp32
inputs and the bf16 tildes.  fp8e4m3 DoubleRow matmuls (0.5
cycles/col, 256-deep contraction) carry both E and the a-side
attention:

  - E = A8'B8 + dA8'B8 + A8'dB8 from host-pre-transposed fp8 pairs
    (dX8 = fp8(X - fp8(X)), first-order residual).  More accurate
    than a bf16 E (logit rms 0.05 vs 0.055) at 0.75x the cycles, and
    no on-chip transposes.
  - a-side: weights pre-normalized into fp8 - Wa8 = fp8(U * (1/s1))
    lies in (0, 1] so fp8's dynamic range holds it; the ones column
    re-derives the denominator from the QUANTIZED weights, so fp8
    rounding of the dominant weight cancels in the ratio.  Values are
    the fp8 pair (B8, dB8).  UT8 = PE-transpose of Wa8.
    4 DR matmuls per group instead of 8 bf16 ones: half the PE time.
  - b-side stays bf16 (U x A_bf16): raw exp values span e^(+-65), far
    beyond fp8 range, and per-column rescaling is not partition-native.
    Measured end-to-end rel err 6.4e-3 vs the 2e-2 gate.
  - Normalize split: Act scales cols 1:512, DVE cols 512:769 (parallel,
    PSUM ring frees sooner).  Reciprocal AFTER chunk2: emitted between
    the chunks it stalls chunk2 on a tile-level WAR hazard.

Schedule: loads issue up front on SP in consumption-deadline order.
Item 0's four E tiles run piece-major across 4 live PSUM tiles
(2 epsum + 2 borrowed apsum).  Steady state: item i+1's E tiles+exps
ride between item i's a-side groups; item i's UT8 thunks ride between
its own b-side groups (after its Wa8 pass completes).  Stores for
items 0-1 go out on the Pool/SWDGE queue (SP is still dispatching
loads); items 2-3 store via SP/HWDGE.

Per-core HBM: in 3.14MB/item (A bf16 + B8/dB8 + 4 transposed fp8),
out 1.57MB/item = 18.9MB (~52us at 360GB/s aggregate) - DMA and PE
(~50us) are balanced at the ridge.
"""

import os as _os

import numpy as np

B, L, D = 32, 512, 768
NCORES = 8
BPC = B // NCORES          # batch items per core
NT = L // 128              # 4 row tiles per matrix
KD = D // 128              # 6 contraction chunks over d
DX = D + 1                 # attention rhs: col 0 = ones, cols 1..768 = data
N1 = 512                   # attention chunk 1: psum cols [s | out 0..510]
C_SHIFT = 120.0            # softmax stabilization shift (valid ~[100, 142])

E_FP8 = int(_os.environ.get("K_E_FP8", "1"))   # E via fp8 DoubleRow + residual
WARMUP = int(_os.environ.get("K_WARMUP", "6"))
EP_BUFS = int(_os.environ.get("K_EP", "2"))
TP_BUFS = int(_os.environ.get("K_TP", "2"))
AP_BUFS = int(_os.environ.get("K_AP", "2"))
AP2_BUFS = int(_os.environ.get("K_AP2", "2"))
OUTP_BUFS = int(_os.environ.get("K_OUTP", "8"))
POOL_STORE_N = int(_os.environ.get("K_PSTORE", "1"))  # items stored via SWDGE
WA8_POOL = int(_os.environ.get("K_WA8POOL", "1"))  # Wa8 scale pass on GpSimd

_CACHE: dict = {}


def _build_bass():
    from contextlib import ExitStack

    import concourse.mybir as mybir
    import concourse.tile as tile
    from concourse import bacc
    from concourse.masks import make_identity

    f32 = mybir.dt.float32
    bf16 = mybir.dt.bfloat16
    f8 = mybir.dt.float8e4
    DR = mybir.MatmulPerfMode.DoubleRow

    nc = bacc.Bacc("TRN2", target_bir_lowering=False, debug=False)

    a_in = nc.dram_tensor("a", [BPC, L, D], bf16, kind="ExternalInput").ap()
    b8u_in = nc.dram_tensor("b8u", [BPC, L, D], f8, kind="ExternalInput").ap()
    db8u_in = nc.dram_tensor("db8u", [BPC, L, D], f8, kind="ExternalInput").ap()
    if E_FP8:
        ha8 = nc.dram_tensor("ha8", [BPC, D, L], f8, kind="ExternalInput").ap()
        hda8 = nc.dram_tensor("hda8", [BPC, D, L], f8, kind="ExternalInput").ap()
        hb8 = nc.dram_tensor("hb8", [BPC, D, L], f8, kind="ExternalInput").ap()
        hdb8 = nc.dram_tensor("hdb8", [BPC, D, L], f8, kind="ExternalInput").ap()
    else:
        ha = nc.dram_tensor("ha", [BPC, D, L], bf16, kind="ExternalInput").ap()
        hb = nc.dram_tensor("hb", [BPC, D, L], bf16, kind="ExternalInput").ap()
    mat_out = nc.dram_tensor("mat", [BPC, L, D], bf16, kind="ExternalOutput").ap()
    mbt_out = nc.dram_tensor("mbt", [BPC, L, D], bf16, kind="ExternalOutput").ap()

    with tile.TileContext(nc) as tc, ExitStack() as ctx:
        singles = ctx.enter_context(tc.tile_pool(name="singles", bufs=1))
        inp = ctx.enter_context(tc.tile_pool(name="inp", bufs=BPC))
        hat = ctx.enter_context(tc.tile_pool(name="hat", bufs=2))
        usb = ctx.enter_context(tc.tile_pool(name="usb", bufs=3))
        outp = ctx.enter_context(tc.tile_pool(name="outp", bufs=OUTP_BUFS))
        stats = ctx.enter_context(tc.tile_pool(name="stats", bufs=16))
        epsum = ctx.enter_context(tc.tile_pool(name="epsum", bufs=EP_BUFS, space="PSUM"))
        tpsum = ctx.enter_context(tc.tile_pool(name="tpsum", bufs=TP_BUFS, space="PSUM"))
        apsum1 = ctx.enter_context(tc.tile_pool(name="apsum1", bufs=AP_BUFS, space="PSUM"))
        apsum2 = ctx.enter_context(tc.tile_pool(name="apsum2", bufs=AP2_BUFS, space="PSUM"))

        ident_f = singles.tile([128, 128], f32, tag="ident_f")
        make_identity(nc, ident_f)
        ident = singles.tile([128, 128], bf16, tag="ident")
        nc.scalar.copy(ident, ident_f)
        neg_shift = singles.tile([128, 1], f32, tag="neg_shift")
        nc.vector.memset(neg_shift, -C_SHIFT)

        # ---- PE p-state warmup: independent of ident (which rides a slow
        # gpsimd iota chain) so it starts immediately.
        if WARMUP:
            wident = singles.tile([128, 128], bf16, tag="wident")
            nc.vector.memset(wident, 0.0)
            wp = apsum1.tile([128, N1], f32, tag="c1")
            for w in range(WARMUP):
                nc.tensor.matmul(
                    wp[:, (w % 4) * 128:(w % 4) * 128 + 128],
                    lhsT=wident, rhs=wident,
                )

        # ---- tiles + load thunks per item (issued in deadline order below)
        inps, hats = [], []
        load_hats, load_ax, load_bx = [], [], []
        for i in range(BPC):
            AX = inp.tile([128, NT, DX], bf16, tag="AX", name=f"AX{i}")
            B8X = inp.tile([128, NT, DX], f8, tag="B8X", name=f"B8X{i}")
            DB8X = inp.tile([128, NT, DX], f8, tag="DB8X", name=f"DB8X{i}")
            nc.gpsimd.memset(AX[:, :, 0:1], 1.0)
            nc.gpsimd.memset(B8X[:, :, 0:1], 1.0)
            nc.gpsimd.memset(DB8X[:, :, 0:1], 0.0)
            if E_FP8:
                HA = hat.tile([128, KD, L], f8, tag="HA", name=f"HA{i}")
                HDA = hat.tile([128, KD, L], f8, tag="HDA", name=f"HDA{i}")
                HB = hat.tile([128, KD, L], f8, tag="HB", name=f"HB{i}")
                HDB = hat.tile([128, KD, L], f8, tag="HDB", name=f"HDB{i}")
                srcs = ((HA, ha8), (HB, hb8), (HDA, hda8), (HDB, hdb8))
            else:
                HA = hat.tile([128, KD, L], bf16, tag="HA", name=f"HA{i}")
                HB = hat.tile([128, KD, L], bf16, tag="HB", name=f"HB{i}")
                HDA = HDB = None
                srcs = ((HA, ha), (HB, hb))

            def mk_hats(i=i, srcs=srcs):
                if i == 0 and len(srcs) == 4:
                    # main tensors in interleaved halves (E main matmuls
                    # start ~1us sooner), residuals in interleaved halves
                    for h in range(2):
                        for dst, src in srcs[:2]:
                            nc.sync.dma_start(
                                out=dst[:, 3 * h:3 * h + 3, :],
                                in_=src[i, 384 * h:384 * h + 384].rearrange(
                                    "(k p) l -> p k l", p=128
                                ),
                            )
                    for h in range(2):
                        for dst, src in srcs[2:]:
                            nc.sync.dma_start(
                                out=dst[:, 3 * h:3 * h + 3, :],
                                in_=src[i, 384 * h:384 * h + 384].rearrange(
                                    "(k p) l -> p k l", p=128
                                ),
                            )
                else:
                    for dst, src in srcs:
                        nc.sync.dma_start(
                            out=dst,
                            in_=src[i].rearrange("(k p) l -> p k l", p=128),
                        )

            def mk_ax(i=i, AX=AX, thirds=(i == 0)):
                src = a_in[i].rearrange("(t p) d -> p t d", p=128)
                if thirds:
                    bounds = [D * q // 3 for q in range(4)]
                    for q in range(3):
                        lo, hi = bounds[q], bounds[q + 1]
                        nc.sync.dma_start(
                            out=AX[:, :, 1 + lo:1 + hi], in_=src[:, :, lo:hi]
                        )
                else:
                    nc.sync.dma_start(out=AX[:, :, 1:DX], in_=src)

            def mk_bx(i=i, B8X=B8X, DB8X=DB8X):
                for dst, src in ((B8X, b8u_in), (DB8X, db8u_in)):
                    nc.sync.dma_start(
                        out=dst[:, :, 1:DX],
                        in_=src[i].rearrange("(t p) d -> p t d", p=128),
                    )

            load_hats.append(mk_hats)
            load_ax.append(mk_ax)
            load_bx.append(mk_bx)
            inps.append((AX, B8X, DB8X))
            hats.append((HA, HDA, HB, HDB))

        # deadline order: hats0, AX0, hats1, BX0, AX1, hats2, BX1, AX2,
        # hats3, BX2, AX3, BX3
        load_hats[0]()
        load_ax[0]()
        load_hats[1]()
        load_bx[0]()
        load_ax[1]()
        load_hats[2]()
        load_bx[1]()
        load_ax[2]()
        load_hats[3]()
        load_bx[2]()
        load_ax[3]()
        load_bx[3]()

        # ---- per-item tiles ----------------------------------------------
        Us, Wa8s, UT8s, s1ps, rs1s = [], [], [], [], []
        for i in range(BPC):
            Us.append(usb.tile([128, NT, L], bf16, tag="U", name=f"U{i}"))
            Wa8s.append(usb.tile([128, NT, L], f8, tag="Wa8", name=f"Wa8{i}"))
            UT8s.append(usb.tile([128, NT, L], f8, tag="UT8", name=f"UT8{i}"))
            s1ps.append(stats.tile([128, NT], f32, tag="s1p", name=f"s1p{i}"))
            rs1s.append(stats.tile([128, NT], f32, tag="rs1", name=f"rs1{i}"))

        def e_terms(i):
            HA, HDA, HB, HDB = hats[i]
            if E_FP8:
                return ((HA, HB), (HDA, HB), (HA, HDB))
            return ((HA, HB),)

        def e_matmul(pe, lt, rt, ta, kp, start, stop):
            if E_FP8:
                nc.tensor.matmul(
                    pe,
                    lhsT=lt[:, 2 * kp:2 * kp + 2, ta * 128:(ta + 1) * 128],
                    rhs=rt[:, 2 * kp:2 * kp + 2, :],
                    start=start, stop=stop, perf_mode=DR,
                    skip_group_check=True,
                )
            else:
                nc.tensor.matmul(
                    pe,
                    lhsT=lt[:, kp, ta * 128:(ta + 1) * 128],
                    rhs=rt[:, kp, :],
                    start=start, stop=stop,
                    skip_group_check=True,
                )

        NKP = (KD // 2) if E_FP8 else KD

        def e_exp(i, ta, pe, half=None):
            if half is None:
                lo, hi = 0, L
            else:
                lo, hi = half * (L // 2), (half + 1) * (L // 2)
            nc.scalar.activation(
                Us[i][:, ta, lo:hi], pe[:, lo:hi],
                mybir.ActivationFunctionType.Exp,
                bias=neg_shift, scale=1.0,
            )

        def wa8_tile(i, ta):
            """1/s1 for tile ta (DVE) -> Wa8 tile = fp8(U * rs1) (Act).
            Each partition holds a different logical row per ta tile, so
            the scale is per-ta."""
            nc.vector.reduce_sum(
                s1ps[i][:, ta:ta + 1], Us[i][:, ta, :],
                axis=mybir.AxisListType.X,
            )
            nc.vector.reciprocal(rs1s[i][:, ta:ta + 1], s1ps[i][:, ta:ta + 1])
            if WA8_POOL:
                nc.gpsimd.tensor_scalar_mul(
                    Wa8s[i][:, ta, :], Us[i][:, ta, :],
                    rs1s[i][:, ta:ta + 1],
                )
            else:
                nc.scalar.activation(
                    Wa8s[i][:, ta, :], Us[i][:, ta, :],
                    mybir.ActivationFunctionType.Copy,
                    scale=rs1s[i][:, ta:ta + 1],
                )

        def e_tile_thunk(i, ta):
            """One E tile (term-major) + exp, for steady-state stages."""
            def th():
                pe = epsum.tile([128, L], f32, tag="pe")
                terms = e_terms(i)
                n = len(terms) * NKP
                j = 0
                for lt, rt in terms:
                    for kp in range(NKP):
                        e_matmul(pe, lt, rt, ta, kp, j == 0, j == n - 1)
                        j += 1
                e_exp(i, ta, pe)
                wa8_tile(i, ta)
            return th

        def ut_thunk(i, tcq):
            def th():
                tp = tpsum.tile([128, L], f8, tag="tp")
                for ta in range(NT):
                    nc.tensor.transpose(
                        tp[:, ta * 128:(ta + 1) * 128],
                        Wa8s[i][:, ta, tcq * 128:(tcq + 1) * 128],
                        ident,
                    )
                nc.vector.tensor_copy(UT8s[i][:, tcq, :], tp)
            return th

        def attn_group(i, side, t):
            AX, B8X, DB8X = inps[i]
            out_dram = mbt_out if side == "b" else mat_out
            # two separate PSUM tiles: the reciprocal + Act normalize of
            # chunk1 overlap chunk2's matmuls (no shared-tile WAR), and
            # each ring frees as soon as its own norm has read it.
            c1 = apsum1.tile([128, N1], f32, tag="c1")
            c2 = apsum2.tile([128, DX - N1], f32, tag="c2")

            def chunk(dst, lo, hi):
                if side == "b":
                    for kc in range(NT):
                        nc.tensor.matmul(
                            dst,
                            lhsT=Us[i][:, kc, t * 128:(t + 1) * 128],
                            rhs=AX[:, kc, lo:hi],
                            start=(kc == 0), stop=(kc == NT - 1),
                        )
                else:
                    j = 0
                    for V in (B8X, DB8X):
                        for q in range(NT // 2):
                            nc.tensor.matmul(
                                dst,
                                lhsT=UT8s[i][:, 2 * q:2 * q + 2,
                                             t * 128:(t + 1) * 128],
                                rhs=V[:, 2 * q:2 * q + 2, lo:hi],
                                start=(j == 0), stop=(j == 3),
                                perf_mode=DR,
                            )
                            j += 1

            chunk(c1, 0, N1)
            r = stats.tile([128, 1], f32, tag="r")
            nc.vector.reciprocal(r, c1[:, 0:1])
            ot = outp.tile([128, D], bf16, tag="ot")
            nc.scalar.activation(
                ot[:, 0:N1 - 1], c1[:, 1:N1],
                mybir.ActivationFunctionType.Copy, scale=r,
            )
            chunk(c2, N1, DX)
            nc.vector.tensor_scalar_mul(ot[:, N1 - 1:D], c2, r)
            rows = slice(t * 128, (t + 1) * 128)
            q = nc.gpsimd if i < POOL_STORE_N else nc.sync
            q.dma_start(out=out_dram[i, rows, :], in_=ot)

        # ---- prologue: item 0's E piece-major across 4 live PSUM tiles ---
        e0 = [
            epsum.tile([128, L], f32, tag="pe", name="e0p0"),
            epsum.tile([128, L], f32, tag="pe", name="e0p1"),
            apsum1.tile([128, N1], f32, tag="c1", name="e0p2"),
            apsum1.tile([128, N1], f32, tag="c1", name="e0p3"),
        ]
        terms0 = e_terms(0)
        npiece = len(terms0) * NKP
        j = 0
        for lt, rt in terms0:
            for kp in range(NKP):
                for ta in range(NT):
                    e_matmul(e0[ta], lt, rt, ta, kp, j == 0, j == npiece - 1)
                j += 1
        # exps in halves: tiles 2,3 first (they hold the borrowed apsum
        # buffers the first b-groups need), then tile 0/1 halves in the
        # order the first b-group's lhsT slices want them.
        for ta, h in ((2, 0), (2, 1), (3, 0), (3, 1),
                      (0, 0), (1, 0), (0, 1), (1, 1)):
            e_exp(0, ta, e0[ta], half=h)
        for ta in range(NT):
            wa8_tile(0, ta)

        # ---- steady state -------------------------------------------------
        pend_ut = [ut_thunk(0, tcq) for tcq in range(NT)]
        for i in range(BPC):
            last = i == BPC - 1
            slots = (0, 0, 2, 2) if i == 0 else (0, 2, 2, 0)
            if not last:
                for t in range(NT):
                    attn_group(i, "b", t)
                    for _ in range(slots[t]):
                        if pend_ut:
                            pend_ut.pop(0)()
                nxt_e = [e_tile_thunk(i + 1, ta) for ta in range(NT)]
                for t in range(NT):
                    attn_group(i, "a", t)
                    if nxt_e:
                        nxt_e.pop(0)()
                pend_ut = [ut_thunk(i + 1, tcq) for tcq in range(NT)]
            else:
                # no fillers left: interleave a-groups into the b-half so
                # the 2-deep PSUM ring never starves PE; "u" slots run the
                # UT8 transposes the a-groups need.
                seq = [("b", 0), "u", "u", ("b", 1), "u", "u",
                       ("b", 2), ("a", 0), ("b", 3), ("a", 1),
                       ("a", 2), ("a", 3)]
                for ent in seq:
                    if ent == "u":
                        if pend_ut:
                            pend_ut.pop(0)()
                    else:
                        attn_group(i, ent[0], ent[1])

    nc.compile()
    return nc


def _get_nc():
    if "nc" not in _CACHE:
        _CACHE["nc"] = _build_bass()
    return _CACHE["nc"]


def host_prep(a_bar, b_bar):
    """Full-batch [B, L, D] fp32 -> per-input dram arrays (full batch)."""
    import ml_dtypes

    bf = ml_dtypes.bfloat16
    f8 = ml_dtypes.float8_e4m3
    a32 = np.asarray(a_bar, dtype=np.float32)
    b32 = np.asarray(b_bar, dtype=np.float32)
    a8 = a32.astype(f8)
    da8 = (a32 - a8.astype(np.float32)).astype(f8)
    b8 = b32.astype(f8)
    db8 = (b32 - b8.astype(np.float32)).astype(f8)
    out = {
        "a": np.ascontiguousarray(a32.astype(bf)),
        "b8u": np.ascontiguousarray(b8),
        "db8u": np.ascontiguousarray(db8),
    }
    if E_FP8:
        out["ha8"] = np.ascontiguousarray(a8.transpose(0, 2, 1))
        out["hda8"] = np.ascontiguousarray(da8.transpose(0, 2, 1))
        out["hb8"] = np.ascontiguousarray(b8.transpose(0, 2, 1))
        out["hdb8"] = np.ascontiguousarray(db8.transpose(0, 2, 1))
    else:
        out["ha"] = np.ascontiguousarray(
            a32.astype(bf).transpose(0, 2, 1)
        )
        out["hb"] = np.ascontiguousarray(
            b32.astype(bf).transpose(0, 2, 1)
        )
    return out


def assemble(x32, t_bf16):
    """m = concat([x, t, x - t, x * t], -1) in fp32."""
    n, l, d = x32.shape
    m = np.empty((n, l, 4 * d), dtype=np.float32)
    t = np.asarray(t_bf16, dtype=np.float32)
    m[:, :, 0:d] = x32
    m[:, :, d:2 * d] = t
    m[:, :, 2 * d:3 * d] = x32 - t
    m[:, :, 3 * d:4 * d] = x32 * t
    return m


def kernel(a_bar, b_bar):
    from concourse import bass_utils

    a32 = np.asarray(a_bar, dtype=np.float32)
    b32 = np.asarray(b_bar, dtype=np.float32)
    full = host_prep(a32, b32)
    nc = _get_nc()
    in_maps = []
    for r in range(NCORES):
        sl = slice(r * BPC, (r + 1) * BPC)
        in_maps.append({k: v[sl] for k, v in full.items()})
    res = bass_utils.run_bass_kernel_spmd(nc, in_maps, core_ids=list(range(NCORES)))

    at = np.concatenate(
        [np.asarray(res.results[r]["mat"]) for r in range(NCORES)], axis=0
    )
    bt = np.concatenate(
        [np.asarray(res.results[r]["mbt"]) for r in range(NCORES)], axis=0
    )
    return assemble(a32, at), assemble(b32, bt)


# revision 20
# speedup vs baseline: 1.0362x; 1.0157x over previous
"""ESIM-style local inference modeling kernel for Trainium2 (Bass/Tile).

Problem (per batch item, B=32, La=Lb=512, D=768, fp32):
    E       = A @ B^T                      [512, 512]
    a_tilde = softmax(E, axis=1) @ B       [512, 768]
    b_tilde = softmax(E, axis=0)^T @ A     [512, 768]
    m_a     = concat([A, a_tilde, A - a_tilde, A * a_tilde], -1)
    m_b     = concat([B, b_tilde, B - b_tilde, B * b_tilde], -1)

Sharding: pure data-parallel, 4 batch items per core across 8 cores.

Strategy (v3): the device computes ONLY a_tilde / b_tilde (bf16); the
concat blocks are assembled host-side in fp32 from the exact fp32
inputs and the bf16 tildes.  fp8e4m3 DoubleRow matmuls (0.5
cycles/col, 256-deep contraction) carry both E and the a-side
attention:

  - E = A8'B8 + dA8'B8 + A8'dB8 from host-pre-transposed fp8 pairs
    (dX8 = fp8(X - fp8(X)), first-order residual).  More accurate
    than a bf16 E (logit rms 0.05 vs 0.055) at 0.75x the cycles, and
    no on-chip transposes.
  - a-side: weights pre-normalized into fp8 - Wa8 = fp8(U * (1/s1))
    lies in (0, 1] so fp8's dynamic range holds it; the ones column
    re-derives the denominator from the QUANTIZED weights, so fp8
    rounding of the dominant weight cancels in the ratio.  Values are
    the fp8 pair (B8, dB8).  UT8 = PE-transpose of Wa8.
    4 DR matmuls per group instead of 8 bf16 ones: half the PE time.
  - b-side stays bf16 (U x A_bf16): raw exp values span e^(+-65), far
    beyond fp8 range, and per-column rescaling is not partition-native.
    Measured end-to-end rel err 6.4e-3 vs the 2e-2 gate.
  - Normalize split: Act scales cols 1:512, DVE cols 512:769 (parallel,
    PSUM ring frees sooner).  Reciprocal AFTER chunk2: emitted between
    the chunks it stalls chunk2 on a tile-level WAR hazard.

Schedule: loads issue up front on SP in consumption-deadline order.
Item 0's four E tiles run piece-major across 4 live PSUM tiles
(2 epsum + 2 borrowed apsum).  Steady state: item i+1's E tiles+exps
ride between item i's a-side groups; item i's UT8 thunks ride between
its own b-side groups (after its Wa8 pass completes).  Stores for
items 0-1 go out on the Pool/SWDGE queue (SP is still dispatching
loads); items 2-3 store via SP/HWDGE.

Per-core HBM: in 3.14MB/item (A bf16 + B8/dB8 + 4 transposed fp8),
out 1.57MB/item = 18.9MB (~52us at 360GB/s aggregate) - DMA and PE
(~50us) are balanced at the ridge.
"""

import os as _os

import numpy as np

B, L, D = 32, 512, 768
NCORES = 8
BPC = B // NCORES          # batch items per core
NT = L // 128              # 4 row tiles per matrix
KD = D // 128              # 6 contraction chunks over d
DX = D + 1                 # attention rhs: col 0 = ones, cols 1..768 = data
N1 = 512                   # attention chunk 1: psum cols [s | out 0..510]
C_SHIFT = 120.0            # softmax stabilization shift (valid ~[100, 142])

E_FP8 = int(_os.environ.get("K_E_FP8", "1"))   # E via fp8 DoubleRow + residual
WARMUP = int(_os.environ.get("K_WARMUP", "6"))
EP_BUFS = int(_os.environ.get("K_EP", "2"))
TP_BUFS = int(_os.environ.get("K_TP", "2"))
AP_BUFS = int(_os.environ.get("K_AP", "2"))
AP2_BUFS = int(_os.environ.get("K_AP2", "2"))
OUTP_BUFS = int(_os.environ.get("K_OUTP", "8"))
POOL_STORE_N = int(_os.environ.get("K_PSTORE", "1"))  # items stored via SWDGE
WA8_POOL = int(_os.environ.get("K_WA8POOL", "1"))  # Wa8 scale pass on GpSimd

_CACHE: dict = {}


def _build_bass():
    from contextlib import ExitStack

    import concourse.mybir as mybir
    import concourse.tile as tile
    from concourse import bacc
    from concourse.masks import make_identity

    f32 = mybir.dt.float32
    bf16 = mybir.dt.bfloat16
    f8 = mybir.dt.float8e4
    DR = mybir.MatmulPerfMode.DoubleRow

    nc = bacc.Bacc("TRN2", target_bir_lowering=False, debug=False)

    a_in = nc.dram_tensor("a", [BPC, L, D], bf16, kind="ExternalInput").ap()
    b8u_in = nc.dram_tensor("b8u", [BPC, L, D], f8, kind="ExternalInput").ap()
    db8u_in = nc.dram_tensor("db8u", [BPC, L, D], f8, kind="ExternalInput").ap()
    if E_FP8:
        ha8 = nc.dram_tensor("ha8", [BPC, D, L], f8, kind="ExternalInput").ap()
        hda8 = nc.dram_tensor("hda8", [BPC, D, L], f8, kind="ExternalInput").ap()
        hb8 = nc.dram_tensor("hb8", [BPC, D, L], f8, kind="ExternalInput").ap()
        hdb8 = nc.dram_tensor("hdb8", [BPC, D, L], f8, kind="ExternalInput").ap()
    else:
        ha = nc.dram_tensor("ha", [BPC, D, L], bf16, kind="ExternalInput").ap()
        hb = nc.dram_tensor("hb", [BPC, D, L], bf16, kind="ExternalInput").ap()
    mat_out = nc.dram_tensor("mat", [BPC, L, D], bf16, kind="ExternalOutput").ap()
    mbt_out = nc.dram_tensor("mbt", [BPC, L, D], bf16, kind="ExternalOutput").ap()

    with tile.TileContext(nc) as tc, ExitStack() as ctx:
        singles = ctx.enter_context(tc.tile_pool(name="singles", bufs=1))
        inp = ctx.enter_context(tc.tile_pool(name="inp", bufs=BPC))
        hat = ctx.enter_context(tc.tile_pool(name="hat", bufs=2))
        usb = ctx.enter_context(tc.tile_pool(name="usb", bufs=3))
        outp = ctx.enter_context(tc.tile_pool(name="outp", bufs=OUTP_BUFS))
        stats = ctx.enter_context(tc.tile_pool(name="stats", bufs=16))
        epsum = ctx.enter_context(tc.tile_pool(name="epsum", bufs=EP_BUFS, space="PSUM"))
        tpsum = ctx.enter_context(tc.tile_pool(name="tpsum", bufs=TP_BUFS, space="PSUM"))
        apsum1 = ctx.enter_context(tc.tile_pool(name="apsum1", bufs=AP_BUFS, space="PSUM"))
        apsum2 = ctx.enter_context(tc.tile_pool(name="apsum2", bufs=AP2_BUFS, space="PSUM"))

        ident_f = singles.tile([128, 128], f32, tag="ident_f")
        make_identity(nc, ident_f)
        ident = singles.tile([128, 128], bf16, tag="ident")
        nc.scalar.copy(ident, ident_f)
        neg_shift = singles.tile([128, 1], f32, tag="neg_shift")
        nc.vector.memset(neg_shift, -C_SHIFT)

        # ---- PE p-state warmup: independent of ident (which rides a slow
        # gpsimd iota chain) so it starts immediately.
        if WARMUP:
            wident = singles.tile([128, 128], bf16, tag="wident")
            nc.vector.memset(wident, 0.0)
            wp = apsum1.tile([128, N1], f32, tag="c1")
            for w in range(WARMUP):
                nc.tensor.matmul(
                    wp[:, (w % 4) * 128:(w % 4) * 128 + 128],
                    lhsT=wident, rhs=wident,
                )

        # ---- tiles + load thunks per item (issued in deadline order below)
        inps, hats = [], []
        load_hats, load_ax, load_bx = [], [], []
        for i in range(BPC):
            AX = inp.tile([128, NT, DX], bf16, tag="AX", name=f"AX{i}")
            B8X = inp.tile([128, NT, DX], f8, tag="B8X", name=f"B8X{i}")
            DB8X = inp.tile([128, NT, DX], f8, tag="DB8X", name=f"DB8X{i}")
            nc.gpsimd.memset(AX[:, :, 0:1], 1.0)
            nc.gpsimd.memset(B8X[:, :, 0:1], 1.0)
            nc.gpsimd.memset(DB8X[:, :, 0:1], 0.0)
            if E_FP8:
                HA = hat.tile([128, KD, L], f8, tag="HA", name=f"HA{i}")
                HDA = hat.tile([128, KD, L], f8, tag="HDA", name=f"HDA{i}")
                HB = hat.tile([128, KD, L], f8, tag="HB", name=f"HB{i}")
                HDB = hat.tile([128, KD, L], f8, tag="HDB", name=f"HDB{i}")
                srcs = ((HA, ha8), (HB, hb8), (HDA, hda8), (HDB, hdb8))
            else:
                HA = hat.tile([128, KD, L], bf16, tag="HA", name=f"HA{i}")
                HB = hat.tile([128, KD, L], bf16, tag="HB", name=f"HB{i}")
                HDA = HDB = None
                srcs = ((HA, ha), (HB, hb))

            def mk_hats(i=i, srcs=srcs):
                if i == 0 and len(srcs) == 4:
                    # main term tensors whole; residuals in interleaved
                    # halves so the E residual matmuls track the arrivals
                    for dst, src in srcs[:2]:
                        nc.sync.dma_start(
                            out=dst,
                            in_=src[i].rearrange("(k p) l -> p k l", p=128),
                        )
                    for h in range(2):
                        for dst, src in srcs[2:]:
                            nc.sync.dma_start(
                                out=dst[:, 3 * h:3 * h + 3, :],
                                in_=src[i, 384 * h:384 * h + 384].rearrange(
                                    "(k p) l -> p k l", p=128
                                ),
                            )
                else:
                    for dst, src in srcs:
                        nc.sync.dma_start(
                            out=dst,
                            in_=src[i].rearrange("(k p) l -> p k l", p=128),
                        )

            def mk_ax(i=i, AX=AX, thirds=(i == 0)):
                src = a_in[i].rearrange("(t p) d -> p t d", p=128)
                if thirds:
                    bounds = [D * q // 3 for q in range(4)]
                    for q in range(3):
                        lo, hi = bounds[q], bounds[q + 1]
                        nc.sync.dma_start(
                            out=AX[:, :, 1 + lo:1 + hi], in_=src[:, :, lo:hi]
                        )
                else:
                    nc.sync.dma_start(out=AX[:, :, 1:DX], in_=src)

            def mk_bx(i=i, B8X=B8X, DB8X=DB8X):
                for dst, src in ((B8X, b8u_in), (DB8X, db8u_in)):
                    nc.sync.dma_start(
                        out=dst[:, :, 1:DX],
                        in_=src[i].rearrange("(t p) d -> p t d", p=128),
                    )

            load_hats.append(mk_hats)
            load_ax.append(mk_ax)
            load_bx.append(mk_bx)
            inps.append((AX, B8X, DB8X))
            hats.append((HA, HDA, HB, HDB))

        # deadline order: hats0, AX0, hats1, BX0, AX1, hats2, BX1, AX2,
        # hats3, BX2, AX3, BX3
        load_hats[0]()
        load_ax[0]()
        load_hats[1]()
        load_bx[0]()
        load_ax[1]()
        load_hats[2]()
        load_bx[1]()
        load_ax[2]()
        load_hats[3]()
        load_bx[2]()
        load_ax[3]()
        load_bx[3]()

        # ---- per-item tiles ----------------------------------------------
        Us, Wa8s, UT8s, s1ps, rs1s = [], [], [], [], []
        for i in range(BPC):
            Us.append(usb.tile([128, NT, L], bf16, tag="U", name=f"U{i}"))
            Wa8s.append(usb.tile([128, NT, L], f8, tag="Wa8", name=f"Wa8{i}"))
            UT8s.append(usb.tile([128, NT, L], f8, tag="UT8", name=f"UT8{i}"))
            s1ps.append(stats.tile([128, NT], f32, tag="s1p", name=f"s1p{i}"))
            rs1s.append(stats.tile([128, NT], f32, tag="rs1", name=f"rs1{i}"))

        def e_terms(i):
            HA, HDA, HB, HDB = hats[i]
            if E_FP8:
                return ((HA, HB), (HDA, HB), (HA, HDB))
            return ((HA, HB),)

        def e_matmul(pe, lt, rt, ta, kp, start, stop):
            if E_FP8:
                nc.tensor.matmul(
                    pe,
                    lhsT=lt[:, 2 * kp:2 * kp + 2, ta * 128:(ta + 1) * 128],
                    rhs=rt[:, 2 * kp:2 * kp + 2, :],
                    start=start, stop=stop, perf_mode=DR,
                    skip_group_check=True,
                )
            else:
                nc.tensor.matmul(
                    pe,
                    lhsT=lt[:, kp, ta * 128:(ta + 1) * 128],
                    rhs=rt[:, kp, :],
                    start=start, stop=stop,
                    skip_group_check=True,
                )

        NKP = (KD // 2) if E_FP8 else KD

        def e_exp(i, ta, pe, half=None):
            if half is None:
                lo, hi = 0, L
            else:
                lo, hi = half * (L // 2), (half + 1) * (L // 2)
            nc.scalar.activation(
                Us[i][:, ta, lo:hi], pe[:, lo:hi],
                mybir.ActivationFunctionType.Exp,
                bias=neg_shift, scale=1.0,
            )

        def wa8_tile(i, ta):
            """1/s1 for tile ta (DVE) -> Wa8 tile = fp8(U * rs1) (Act).
            Each partition holds a different logical row per ta tile, so
            the scale is per-ta."""
            nc.vector.reduce_sum(
                s1ps[i][:, ta:ta + 1], Us[i][:, ta, :],
                axis=mybir.AxisListType.X,
            )
            nc.vector.reciprocal(rs1s[i][:, ta:ta + 1], s1ps[i][:, ta:ta + 1])
            if WA8_POOL:
                nc.gpsimd.tensor_scalar_mul(
                    Wa8s[i][:, ta, :], Us[i][:, ta, :],
                    rs1s[i][:, ta:ta + 1],
                )
            else:
                nc.scalar.activation(
                    Wa8s[i][:, ta, :], Us[i][:, ta, :],
                    mybir.ActivationFunctionType.Copy,
                    scale=rs1s[i][:, ta:ta + 1],
                )

        def e_tile_thunk(i, ta):
            """One E tile (term-major) + exp, for steady-state stages."""
            def th():
                pe = epsum.tile([128, L], f32, tag="pe")
                terms = e_terms(i)
                n = len(terms) * NKP
                j = 0
                for lt, rt in terms:
                    for kp in range(NKP):
                        e_matmul(pe, lt, rt, ta, kp, j == 0, j == n - 1)
                        j += 1
                e_exp(i, ta, pe)
                wa8_tile(i, ta)
            return th

        def ut_thunk(i, tcq):
            def th():
                tp = tpsum.tile([128, L], f8, tag="tp")
                for ta in range(NT):
                    nc.tensor.transpose(
                        tp[:, ta * 128:(ta + 1) * 128],
                        Wa8s[i][:, ta, tcq * 128:(tcq + 1) * 128],
                        ident,
                    )
                nc.vector.tensor_copy(UT8s[i][:, tcq, :], tp)
            return th

        def attn_group(i, side, t):
            AX, B8X, DB8X = inps[i]
            out_dram = mbt_out if side == "b" else mat_out
            # two separate PSUM tiles: the reciprocal + Act normalize of
            # chunk1 overlap chunk2's matmuls (no shared-tile WAR), and
            # each ring frees as soon as its own norm has read it.
            c1 = apsum1.tile([128, N1], f32, tag="c1")
            c2 = apsum2.tile([128, DX - N1], f32, tag="c2")

            def chunk(dst, lo, hi):
                if side == "b":
                    for kc in range(NT):
                        nc.tensor.matmul(
                            dst,
                            lhsT=Us[i][:, kc, t * 128:(t + 1) * 128],
                            rhs=AX[:, kc, lo:hi],
                            start=(kc == 0), stop=(kc == NT - 1),
                        )
                else:
                    j = 0
                    for V in (B8X, DB8X):
                        for q in range(NT // 2):
                            nc.tensor.matmul(
                                dst,
                                lhsT=UT8s[i][:, 2 * q:2 * q + 2,
                                             t * 128:(t + 1) * 128],
                                rhs=V[:, 2 * q:2 * q + 2, lo:hi],
                                start=(j == 0), stop=(j == 3),
                                perf_mode=DR,
                            )
                            j += 1

            chunk(c1, 0, N1)
            r = stats.tile([128, 1], f32, tag="r")
            nc.vector.reciprocal(r, c1[:, 0:1])
            ot = outp.tile([128, D], bf16, tag="ot")
            nc.scalar.activation(
                ot[:, 0:N1 - 1], c1[:, 1:N1],
                mybir.ActivationFunctionType.Copy, scale=r,
            )
            chunk(c2, N1, DX)
            nc.vector.tensor_scalar_mul(ot[:, N1 - 1:D], c2, r)
            rows = slice(t * 128, (t + 1) * 128)
            q = nc.gpsimd if i < POOL_STORE_N else nc.sync
            q.dma_start(out=out_dram[i, rows, :], in_=ot)

        # ---- prologue: item 0's E piece-major across 4 live PSUM tiles ---
        e0 = [
            epsum.tile([128, L], f32, tag="pe", name="e0p0"),
            epsum.tile([128, L], f32, tag="pe", name="e0p1"),
            apsum1.tile([128, N1], f32, tag="c1", name="e0p2"),
            apsum1.tile([128, N1], f32, tag="c1", name="e0p3"),
        ]
        terms0 = e_terms(0)
        npiece = len(terms0) * NKP
        j = 0
        for lt, rt in terms0:
            for kp in range(NKP):
                for ta in range(NT):
                    e_matmul(e0[ta], lt, rt, ta, kp, j == 0, j == npiece - 1)
                j += 1
        # exps in halves: tiles 2,3 first (they hold the borrowed apsum
        # buffers the first b-groups need), then tile 0/1 halves in the
        # order the first b-group's lhsT slices want them.
        for ta, h in ((2, 0), (2, 1), (3, 0), (3, 1),
                      (0, 0), (1, 0), (0, 1), (1, 1)):
            e_exp(0, ta, e0[ta], half=h)
        for ta in range(NT):
            wa8_tile(0, ta)

        # ---- steady state -------------------------------------------------
        pend_ut = [ut_thunk(0, tcq) for tcq in range(NT)]
        for i in range(BPC):
            last = i == BPC - 1
            slots = (0, 0, 2, 2) if i == 0 else (0, 2, 2, 0)
            if not last:
                for t in range(NT):
                    attn_group(i, "b", t)
                    for _ in range(slots[t]):
                        if pend_ut:
                            pend_ut.pop(0)()
                nxt_e = [e_tile_thunk(i + 1, ta) for ta in range(NT)]
                for t in range(NT):
                    attn_group(i, "a", t)
                    if nxt_e:
                        nxt_e.pop(0)()
                pend_ut = [ut_thunk(i + 1, tcq) for tcq in range(NT)]
            else:
                # no fillers left: interleave a-groups into the b-half so
                # the 2-deep PSUM ring never starves PE; "u" slots run the
                # UT8 transposes the a-groups need.
                seq = [("b", 0), "u", "u", ("b", 1), "u", "u",
                       ("b", 2), ("a", 0), ("b", 3), ("a", 1),
                       ("a", 2), ("a", 3)]
                for ent in seq:
                    if ent == "u":
                        if pend_ut:
                            pend_ut.pop(0)()
                    else:
                        attn_group(i, ent[0], ent[1])

    nc.compile()
    return nc


def _get_nc():
    if "nc" not in _CACHE:
        _CACHE["nc"] = _build_bass()
    return _CACHE["nc"]


def host_prep(a_bar, b_bar):
    """Full-batch [B, L, D] fp32 -> per-input dram arrays (full batch)."""
    import ml_dtypes

    bf = ml_dtypes.bfloat16
    f8 = ml_dtypes.float8_e4m3
    a32 = np.asarray(a_bar, dtype=np.float32)
    b32 = np.asarray(b_bar, dtype=np.float32)
    a8 = a32.astype(f8)
    da8 = (a32 - a8.astype(np.float32)).astype(f8)
    b8 = b32.astype(f8)
    db8 = (b32 - b8.astype(np.float32)).astype(f8)
    out = {
        "a": np.ascontiguousarray(a32.astype(bf)),
        "b8u": np.ascontiguousarray(b8),
        "db8u": np.ascontiguousarray(db8),
    }
    if E_FP8:
        out["ha8"] = np.ascontiguousarray(a8.transpose(0, 2, 1))
        out["hda8"] = np.ascontiguousarray(da8.transpose(0, 2, 1))
        out["hb8"] = np.ascontiguousarray(b8.transpose(0, 2, 1))
        out["hdb8"] = np.ascontiguousarray(db8.transpose(0, 2, 1))
    else:
        out["ha"] = np.ascontiguousarray(
            a32.astype(bf).transpose(0, 2, 1)
        )
        out["hb"] = np.ascontiguousarray(
            b32.astype(bf).transpose(0, 2, 1)
        )
    return out


def assemble(x32, t_bf16):
    """m = concat([x, t, x - t, x * t], -1) in fp32."""
    n, l, d = x32.shape
    m = np.empty((n, l, 4 * d), dtype=np.float32)
    t = np.asarray(t_bf16, dtype=np.float32)
    m[:, :, 0:d] = x32
    m[:, :, d:2 * d] = t
    m[:, :, 2 * d:3 * d] = x32 - t
    m[:, :, 3 * d:4 * d] = x32 * t
    return m


def kernel(a_bar, b_bar):
    from concourse import bass_utils

    a32 = np.asarray(a_bar, dtype=np.float32)
    b32 = np.asarray(b_bar, dtype=np.float32)
    full = host_prep(a32, b32)
    nc = _get_nc()
    in_maps = []
    for r in range(NCORES):
        sl = slice(r * BPC, (r + 1) * BPC)
        in_maps.append({k: v[sl] for k, v in full.items()})
    res = bass_utils.run_bass_kernel_spmd(nc, in_maps, core_ids=list(range(NCORES)))

    at = np.concatenate(
        [np.asarray(res.results[r]["mat"]) for r in range(NCORES)], axis=0
    )
    bt = np.concatenate(
        [np.asarray(res.results[r]["mbt"]) for r in range(NCORES)], axis=0
    )
    return assemble(a32, at), assemble(b32, bt)
